# revision 54
# baseline (speedup 1.0000x reference)
import sys
import types

import numpy as np
from contextlib import ExitStack

try:
    import antenv.axon_hooks  # noqa: F401
except ImportError:
    _m = types.ModuleType("antenv.axon_hooks")
    _m._HOOK = None

    def _set_hook(h, _m=_m):
        _m._HOOK = h

    def _get_hook(_m=_m):
        return _m._HOOK

    _m.set_axon_ntff_profile_hook = _set_hook
    _m.get_axon_ntff_profile_hook = _get_hook
    sys.modules["antenv.axon_hooks"] = _m
    try:
        import antenv

        antenv.axon_hooks = _m
    except ImportError:
        pass

import concourse.bass as bass
import concourse.bacc as bacc
import concourse.tile as tile
from concourse import mybir
from concourse.bass_utils import run_bass_kernel_spmd
from concourse.masks import make_identity

F32 = mybir.dt.float32
F16 = mybir.dt.float16
AF = mybir.ActivationFunctionType
OP = mybir.AluOpType

B, S, D, M = 32, 2048, 1024, 1024
NC = 8
BP = B // NC          # batches per core = 4
NT = 16               # X DMA tiles per core (each [128, 4*1024] f16)
SUB = 4               # 128-row subtiles per X DMA tile
LN_EPS = 1e-5
SC = float(2 ** 20)   # scaling so the per-row gradient c stays in f16 normal range

# packed broadcast-row order (all f32, one DRAM tensor)
ROWS = ("bk", "b0", "b1", "g0", "g1", "lb0", "lb1", "bfv", "buv", "wvs")

LAST_RESULT = None    # test.py reads exec_time_ns from here


def _build(eta_f: float, theta_f: float, bvs_pre_sc: float, sc_val: float):
    nc = bacc.Bacc("TRN2", target_bir_lowering=False)
    d = nc.declare_dram_parameter
    x_d = d("x", [NT * 128, SUB * (D + 1)], F16, False)
    mem_d = d("mem", [BP, M], F32, False)
    mom_d = d("mom", [BP, M], F32, False)
    # square weights packed [128, 8*1024]: cols 1024k:1024(k+1) = W[128k:128(k+1), :]
    wkT_d = d("wkT", [128, 8 * 1024], F16, False)
    wk_d = d("wk", [128, 8 * 1024], F16, False)
    w0_d = d("w0", [128, 8 * 1024], F16, False)
    w1_d = d("w1", [128, 8 * 1024], F16, False)
    w0T_d = d("w0T", [128, 8 * 1024], F16, False)
    w1T_d = d("w1T", [128, 8 * 1024], F16, False)
    wf_d = d("wf", [128, 16 * 1024], F16, False)
    wu_d = d("wu", [128, 16 * 1024], F16, False)
    rows_d = d("rows", [BP, len(ROWS) * M], F32, False)
    bkT_d = d("bkT", [128, 8], F16, False)
    outp_d = d("out_p", [BP, M], F32, True)
    outm_d = d("out_m", [BP, M], F32, True)

    with tile.TileContext(nc) as tc, ExitStack() as ctx:
        keep = ctx.enter_context(tc.tile_pool(name="keep", bufs=1))
        temps = ctx.enter_context(tc.tile_pool(name="temps", bufs=6))
        sc = ctx.enter_context(tc.tile_pool(name="sc", bufs=12))
        wch = ctx.enter_context(tc.tile_pool(name="wch", bufs=2))
        tp = ctx.enter_context(tc.tile_pool(name="tp", bufs=3))

        def kt(tag, shape=(BP, M), dt=F32):
            return keep.tile(list(shape), dt, tag=tag, name=tag)

        def tmp():
            return temps.tile([BP, M], F32, tag="tmp", name="tmp")

        def sct():
            return sc.tile([BP, 1], F32, tag="sc", name="sc")

        ident = kt("ident", (128, 128))
        make_identity(nc, ident[:])
        epsc = kt("epsc", (BP, 1))
        nc.gpsimd.memset(epsc[:], LN_EPS)

        # ---- cached / streamed weights (halves so chunk 0 lands early) ----
        w0ca = kt("w0ca", (128, 4 * 1024), F16)
        nc.sync.dma_start(w0ca[:], w0_d[:, 0:4 * 1024])
        w0cb = kt("w0cb", (128, 4 * 1024), F16)
        nc.sync.dma_start(w0cb[:], w0_d[:, 4 * 1024:8 * 1024])
        w0c = [w0ca, w0cb]

        # broadcast rows: host replicates 4x, one direct DMA
        cbt = kt("cbt", (BP, len(ROWS) * M))
        nc.sync.dma_start(cbt[:], rows_d[:])
        cb = {n: cbt[:, i * M:(i + 1) * M] for i, n in enumerate(ROWS)}

        mem_sb = kt("mem")
        nc.sync.dma_start(mem_sb[:], mem_d[:])
        mom_sb = kt("mom")
        nc.sync.dma_start(mom_sb[:], mom_d[:])
        bkT = kt("bkT", (128, 8), F16)
        nc.sync.dma_start(bkT[:], bkT_d[:])

        def stream_w(dram, off=0):
            t = wch.tile([128, 4 * 1024], F16, tag="wch")
            nc.sync.dma_start(t[:], dram[:, off:off + 4 * 1024])
            return t

        def stream_sq(dram):
            return [stream_w(dram, 0), stream_w(dram, 4 * 1024)]

        def transpose_4(src, ps_tp, tag, dst_pool=None, cast_scale=None):
            # [4, 1024] f32 -> f16 [128, 32]; chunk k lives at cols 4k:4k+4
            pool = dst_pool if dst_pool is not None else tp
            dst = pool.tile([128, 4 * (M // 128)], F16, tag=tag)
            for k in range(M // 128):
                pt = ps_tp.tile([128, BP], F32, tag="pt")
                nc.tensor.transpose(pt[:], src[:, 128 * k:128 * (k + 1)],
                                    ident[0:BP, 0:BP])
                if cast_scale is None:
                    nc.scalar.copy(dst[:, 4 * k:4 * k + 4], pt[:])
                else:
                    nc.scalar.activation(dst[:, 4 * k:4 * k + 4], pt[:],
                                         AF.Copy, scale=cast_scale)
            return dst

        def mm_sb(lhsT_ap_fn, w_tiles, nk, ps_mm, evict):
            # out[b, n] = sum_k lhs[b, k] * W[k, n]; rhs views into resident
            # SBUF tiles (w_tiles[k] -> (tile, col_off) for chunk k)
            pz0 = ps_mm.tile([BP, 512], F32, tag="pz0")
            pz1 = ps_mm.tile([BP, 512], F32, tag="pz1")
            for k in range(nk):
                wt, off = w_tiles(k)
                nc.tensor.matmul(pz0[:], lhsT_ap_fn(k), wt[:, off:off + 512],
                                 start=(k == 0), stop=(k == nk - 1))
                nc.tensor.matmul(pz1[:], lhsT_ap_fn(k), wt[:, off + 512:off + 1024],
                                 start=(k == 0), stop=(k == nk - 1))
            evict(0, pz0)
            evict(1, pz1)

        def sq_tiles(t):
            if isinstance(t, list):
                return lambda k: (t[k // 4], 1024 * (k % 4))
            return lambda k: (t, 1024 * k)

        def layer_forward(h_sb, w_tile, b_b, g_b, lb_b, ps_tp, ps_mm, li,
                          hT_tag=None, hT_pool=None, save=False):
            hT = transpose_4(h_sb, ps_tp, hT_tag or f"hT{li}", dst_pool=hT_pool)
            z_sb = tmp()

            def ev(half, pz):
                nc.vector.tensor_add(z_sb[:, 512 * half:512 * half + 512], pz[:],
                                     b_b[:, 512 * half:512 * half + 512])

            mm_sb(lambda k: hT[:, 4 * k:4 * k + 4], sq_tiles(w_tile), 8, ps_mm, ev)

            ssum = sct()
            nc.vector.tensor_reduce(ssum[:], z_sb[:], mybir.AxisListType.X, OP.add)
            nmean = sct()
            nc.scalar.mul(nmean[:], ssum[:], -1.0 / M)
            sq = tmp()
            vs = sct()
            nc.scalar.activation(sq[:], z_sb[:], AF.Square, bias=nmean[:],
                                 accum_out=vs[:])
            std = sct()
            nc.scalar.activation(std[:], vs[:], AF.Sqrt, bias=epsc[:],
                                 scale=1.0 / M)
            rstd = kt(f"rstd{li}", (BP, 1)) if save else sct()
            nc.vector.reciprocal(rstd[:], std[:])
            xhat = kt(f"xhat{li}") if save else tmp()
            nc.vector.tensor_scalar(xhat[:], z_sb[:], nmean[:], rstd[:],
                                    OP.add, OP.mult)
            yt = tmp()
            nc.vector.tensor_mul(yt[:], xhat[:], g_b[:])
            y_sb = kt(f"y{li}") if save else tmp()
            nc.vector.tensor_add(y_sb[:], yt[:], lb_b[:])
            h_next = tmp()
            nc.scalar.activation(h_next[:], y_sb[:], AF.Silu)
            return h_next, hT, xhat, y_sb, rstd

        # ---------- Phase A: forward MLP(mem) -> mo, then u, a, beta ----------
        with tc.tile_pool(name="pstp_a", bufs=2, space="PSUM") as ps_tp, \
             tc.tile_pool(name="psmm_a", bufs=2, space="PSUM") as ps_mm, \
             tc.tile_pool(name="rowp", bufs=2) as rowp:
            w1_sa = stream_sq(w1_d)
            wkT_sb = stream_sq(wkT_d)

            h1, memT, xhat0, y0, rstd0 = layer_forward(
                mem_sb, w0c, cb["b0"], cb["g0"], cb["lb0"], ps_tp, ps_mm, 0,
                hT_tag="memT", hT_pool=keep, save=True)
            mo, _, xhat1, y1, rstd1 = layer_forward(
                h1, w1_sa, cb["b1"], cb["g1"], cb["lb1"], ps_tp, ps_mm, 1,
                save=True)

            # kappa = mo . bk via PE (moT chunks x bkT cols)
            moT = transpose_4(mo, ps_tp, "moT")
            kap = kt("kap", (BP, 1))
            kpz = ps_mm.tile([BP, 1], F32, tag="kpz")
            for k in range(8):
                nc.tensor.matmul(kpz[:], moT[:, 4 * k:4 * k + 4],
                                 bkT[:, k:k + 1], start=(k == 0), stop=(k == 7))
            nc.scalar.copy(kap[:], kpz[:])
            # u = mo @ WkT, pre-scaled: us = u * SC/(B*S)
            us = tmp()

            def ev_u(half, pz):
                nc.scalar.activation(us[:, 512 * half:512 * half + 512], pz[:],
                                     AF.Copy, scale=sc_val / (B * S))

            mm_sb(lambda k: moT[:, 4 * k:4 * k + 4], sq_tiles(wkT_sb), 8, ps_mm, ev_u)

            # abrow[:, 0:D] = a' = u*SC/(B*S) - wvs*SC/(B*S*M)   (wvs pre-scaled on host)
            # abrow[:, D]   = beta' = SC*(kappa/(B*S) - bvs/(B*S*M))
            abrow16 = kt("abrow16", (BP, D + 1), F16)
            nc.vector.tensor_sub(abrow16[:, 0:D], us[:], cb["wvs"])
            nc.scalar.activation(abrow16[:, D:D + 1], kap[:], AF.Copy,
                                 bias=-bvs_pre_sc, scale=sc_val / (B * S))

            # partition_broadcast input must start at partition 0 -> DMA-stage
            # (scalar queue: keeps the sync queue free for the X stream)
            a_bc = []
            for b in range(BP):
                row = rowp.tile([1, D + 1], F16, tag="row", name=f"row{b}")
                nc.scalar.dma_start(row[:], abrow16[b:b + 1, :])
                ab = kt(f"abc{b}", (128, D + 1), F16)
                nc.gpsimd.partition_broadcast(ab[:], row[:])
                a_bc.append(ab)

        # ---------- Phase B: stream X (f16), c' = SC*(X.a + beta) ----------
        gx_sb = kt("gx")
        xsum_sb = kt("xsum")
        csum_sb = kt("csum", (BP, 1))
        with tc.tile_pool(name="pa", bufs=2, space="PSUM") as pa_p, \
             tc.tile_pool(name="pb", bufs=2, space="PSUM") as pb_p, \
             tc.tile_pool(name="pc", bufs=2, space="PSUM") as pc_p, \
             tc.tile_pool(name="xt", bufs=3) as xt_p, \
             tc.tile_pool(name="ctp", bufs=3) as ct_p, \
             tc.tile_pool(name="c32p", bufs=3) as c32_p, \
             tc.tile_pool(name="scrp", bufs=4) as scr_p, \
             tc.tile_pool(name="stg", bufs=1) as stg_p:
            for b in range(BP):
                pa = pa_p.tile([2, 512], F32, tag="pa")
                pb = pb_p.tile([2, 512], F32, tag="pb")
                pc = pc_p.tile([2 * SUB, 2 * SUB], F32, tag="pc")
                for t in range(NT // BP):
                    di = b * (NT // BP) + t
                    xt = xt_p.tile([128, SUB * (D + 1)], F16, tag="xt")
                    # alternate the two HWDGE rings to pipeline fixed costs
                    dq = nc.sync if di % 2 == 0 else nc.scalar
                    dq.dma_start(xt[:], x_d[di * 128:(di + 1) * 128, :])
                    ct = ct_p.tile([128, 2 * SUB], F16, tag="ct")
                    nc.any.memset(ct[:], 1.0)
                    c32 = c32_p.tile([128, SUB], F32, tag="c32")
                    for j in range(SUB):
                        subf = xt[:, j * (D + 1):(j + 1) * (D + 1)]
                        scr = scr_p.tile([128, D + 1], F16, tag="scr")
                        nc.vector.tensor_mul(scr[:], subf, a_bc[b][:])
                        scr2 = scr_p.tile([128, D + 1], F16, tag="scr")
                        nc.scalar.activation(scr2[:], scr[:], AF.Copy,
                                             accum_out=c32[:, j:j + 1])
                    nc.scalar.copy(ct[:, 1:2 * SUB:2], c32[:])
                    for j in range(SUB):
                        st = t * SUB + j
                        sub = xt[:, j * (D + 1):j * (D + 1) + D]
                        lt = ct[:, 2 * j:2 * j + 2]
                        fl = (st == 0)
                        ll = (st == 4 * SUB - 1)
                        nc.tensor.matmul(pa[:], lt, sub[:, 0:512], start=fl, stop=ll)
                        nc.tensor.matmul(pb[:], lt, sub[:, 512:1024], start=fl, stop=ll)
                    # csum via one [8,8] matmul per tile; row 0 odd cols hold
                    # per-subtile csums
                    nc.tensor.matmul(pc[:], ct[:], ct[:], start=(t == 0),
                                     stop=(t == NT // BP - 1))
                # lhsT rows: p=0 -> ones, p=1 -> c'
                stage = stg_p.tile([2, D + 2 * SUB + 1], F32, tag="stage")
                nc.scalar.copy(stage[:, 0:512], pa[:])
                nc.scalar.copy(stage[:, 512:1024], pb[:])
                nc.scalar.copy(stage[0:1, 1024:1024 + 2 * SUB], pc[0:1, :])
                nc.vector.tensor_reduce(
                    stage[0:1, D + 2 * SUB:D + 2 * SUB + 1],
                    stage[0:1, D + 1:D + 2 * SUB:2],
                    mybir.AxisListType.X, OP.add)
                nc.gpsimd.dma_start(xsum_sb[b:b + 1, :], stage[0:1, 0:D])
                nc.gpsimd.dma_start(gx_sb[b:b + 1, :], stage[1:2, 0:D])
                nc.gpsimd.dma_start(csum_sb[b:b + 1, 0:1],
                                    stage[0:1, D + 2 * SUB:D + 2 * SUB + 1])

        # ---------- Phase C: dmo, backward, gates, update, output MLP ----------
        with tc.tile_pool(name="pstp_c", bufs=2, space="PSUM") as ps_tp, \
             tc.tile_pool(name="psmm_c", bufs=2, space="PSUM") as ps_mm:
            # stream loads issued in consumption order; wch bufs=3 keeps the
            # DMA queue busy end-to-end
            wk_sb = stream_sq(wk_d)
            w1T_sb = stream_sq(w1T_d)
            w0T_sb = stream_sq(w0T_d)
            wf_sb = [stream_w(wf_d, i * 4 * 1024) for i in range(4)]
            wu_sb = [stream_w(wu_d, i * 4 * 1024) for i in range(4)]
            w1_sc = stream_sq(w1_d)



            # dmo' = SC*dmo = gx' @ Wk + csum' * bk; the 1/SC unscale is
            # deferred to the theta multiply (backward is linear in dmo)
            bkc = tmp()
            nc.vector.tensor_scalar(bkc[:], cb["bk"], csum_sb[:, 0:1], None,
                                    OP.mult)
            gxT = transpose_4(gx_sb, ps_tp, "gxT")
            dmo = kt("dmo")

            def ev_dmo(half, pz):
                nc.vector.tensor_add(dmo[:, 512 * half:512 * half + 512], pz[:],
                                     bkc[:, 512 * half:512 * half + 512])

            mm_sb(lambda k: gxT[:, 4 * k:4 * k + 4], sq_tiles(wk_sb), 8, ps_mm, ev_dmo)

            # backward through the 2-layer MLP
            dcur = dmo
            for i in (1, 0):
                y_i = y1 if i == 1 else y0
                xh_i = xhat1 if i == 1 else xhat0
                rs_i = rstd1 if i == 1 else rstd0
                g_b = cb["g1"] if i == 1 else cb["g0"]
                wT_sb = w1T_sb if i == 1 else w0T_sb

                t4 = tmp()
                nc.scalar.activation(t4[:], y_i[:], AF.Derivative_silu)
                dy = tmp()
                nc.vector.tensor_mul(dy[:], dcur[:], t4[:])
                dxh = tmp()
                nc.vector.tensor_mul(dxh[:], dy[:], g_b[:])

                rsum = sct()
                nc.vector.tensor_reduce(rsum[:], dxh[:], mybir.AxisListType.X, OP.add)
                nm1 = sct()
                nc.scalar.mul(nm1[:], rsum[:], -1.0 / M)
                junk = tmp()
                nc.vector.tensor_mul(junk[:], dxh[:], xh_i[:])
                rs2 = sct()
                junk2 = tmp()
                nc.scalar.activation(junk2[:], junk[:], AF.Copy, accum_out=rs2[:])
                nmh = sct()
                nc.scalar.mul(nmh[:], rs2[:], -1.0 / M)
                t5 = tmp()
                nc.vector.tensor_scalar(t5[:], xh_i[:], nmh[:], nm1[:],
                                        OP.mult, OP.add)
                t6 = tmp()
                nc.vector.tensor_add(t6[:], dxh[:], t5[:])
                dz = tmp()
                nc.vector.tensor_scalar(dz[:], t6[:], rs_i[:], None, OP.mult)

                # layer-0 dz is ~SC*rstd0*rstd1 scaled; shrink 2^-10 at the
                # f16 cast to stay under f16 max (compensated in theta mul)
                dzT = transpose_4(dz, ps_tp, f"dzT{i}",
                                  cast_scale=(None if i == 1 else 2.0 ** -10))
                dnext = kt(f"dh{i}")

                def ev_dh(half, pz, _dst=dnext):
                    nc.scalar.copy(_dst[:, 512 * half:512 * half + 512], pz[:])

                mm_sb(lambda k: dzT[:, 4 * k:4 * k + 4], sq_tiles(wT_sb), 8,
                      ps_mm, ev_dh)
                dcur = dnext
            surprise = dcur

            # gates: gate_in = [pooled | mem]; 1/S folded into the f16 cast
            pooledT = transpose_4(xsum_sb, ps_tp, "pooledT",
                                  cast_scale=1.0 / S)

            def gate_lhsT(k):
                if k < 8:
                    return pooledT[:, 4 * k:4 * k + 4]
                return memT[:, 4 * (k - 8):4 * (k - 8) + 4]

            def make_gate(w_tiles, bias_b, tag):
                g_sb = kt(tag)

                def gw(k):
                    return (w_tiles[k // 4], 1024 * (k % 4))

                def ev(half, pz):
                    tt = tmp()
                    nc.vector.tensor_add(tt[:, 0:512], pz[:],
                                         bias_b[:, 512 * half:512 * half + 512])
                    nc.scalar.activation(g_sb[:, 512 * half:512 * half + 512],
                                         tt[:, 0:512], AF.Sigmoid)

                mm_sb(gate_lhsT, gw, 16, ps_mm, ev)
                return g_sb

            forget_g = make_gate(wf_sb, cb["bfv"], "fgate")
            update_g = make_gate(wu_sb, cb["buv"], "ugate")

            # new_momentum = eta*mom + (theta/SC)*surprise'  (deferred unscale)
            ta = tmp()
            nc.vector.tensor_scalar(ta[:], mom_sb[:], eta_f, None, OP.mult)
            tb = tmp()
            nc.vector.tensor_scalar(tb[:], surprise[:],
                                    theta_f * (2.0 ** 10) / sc_val, None,
                                    OP.mult)
            nm_sb = tmp()
            nc.vector.tensor_add(nm_sb[:], ta[:], tb[:])

            # new_memory = (1-forget)*mem + update*new_momentum
            tc1 = tmp()
            nc.vector.tensor_mul(tc1[:], forget_g[:], mem_sb[:])
            tc2 = tmp()
            nc.vector.tensor_sub(tc2[:], mem_sb[:], tc1[:])
            tc3 = tmp()
            nc.vector.tensor_mul(tc3[:], update_g[:], nm_sb[:])
            newmem = kt("newmem")
            nc.vector.tensor_add(newmem[:], tc2[:], tc3[:])

            # processed = MLP(new_memory), weights cached in SBUF
            p1, _, _, _, _ = layer_forward(newmem, w0c, cb["b0"], cb["g0"],
                                           cb["lb0"], ps_tp, ps_mm, 0,
                                           hT_tag="nmT")
            proc, _, _, _, _ = layer_forward(p1, w1_sc, cb["b1"], cb["g1"],
                                             cb["lb1"], ps_tp, ps_mm, 1,
                                             hT_tag="p1T")

            nc.sync.dma_start(outp_d[:], proc[:])
            nc.sync.dma_start(outm_d[:], newmem[:])

    nc.finalize()
    return nc


def _pack_sq(W):
    # [1024, 1024] -> [128, 8*1024] f16, chunk k = W[128k:128(k+1), :]
    return np.ascontiguousarray(
        W.reshape(8, 128, 1024).transpose(1, 0, 2).reshape(128, 8 * 1024)
    ).astype(np.float16)


def _pack_gate(W):
    # [2048, 1024] -> [128, 16*1024] f16
    return np.ascontiguousarray(
        W.reshape(16, 128, 1024).transpose(1, 0, 2).reshape(128, 16 * 1024)
    ).astype(np.float16)


def _prep(inputs):
    f = lambda k: np.ascontiguousarray(np.asarray(inputs[k], dtype=np.float32))
    X = f("inputs")
    mem = f("memory_state")
    mom = f("momentum_state")
    Wk, bk = f("Wk"), f("bk")
    Wv, bv = f("Wv"), f("bv")
    mem_W, mem_b = f("mem_W"), f("mem_b")
    ln_g, ln_b = f("ln_g"), f("ln_b")
    Wf, Wu = f("Wf"), f("Wu")
    bfv, buv = f("bf"), f("bu")
    eta_f = float(np.asarray(inputs["eta"]).reshape(-1)[0])
    theta_f = float(np.asarray(inputs["theta"]).reshape(-1)[0])

    # keep c' = SC*c in f16 normal range: mem==0 makes c ~1e-7, else ~1e-4
    sc_val = float(2 ** 20) if float(np.abs(mem).max()) < 1e-6 else float(2 ** 10)
    bvs_pre_sc = float(bv.sum()) * sc_val / (B * S * M)
    wvs_sc = (Wv.sum(axis=1) * sc_val / (B * S * M)).astype(np.float32)

    nc = _build(eta_f, theta_f, bvs_pre_sc, sc_val)

    rowvals = {
        "bk": bk, "b0": mem_b[0], "b1": mem_b[1], "g0": ln_g[0], "g1": ln_g[1],
        "lb0": ln_b[0], "lb1": ln_b[1], "bfv": bfv, "buv": buv, "wvs": wvs_sc,
    }
    rows = np.concatenate([rowvals[n].reshape(-1) for n in ROWS]).astype(
        np.float32).reshape(1, len(ROWS) * M)
    rows = np.ascontiguousarray(np.broadcast_to(rows, (BP, len(ROWS) * M)))

    shared = {
        "wk": _pack_sq(Wk),
        "wkT": _pack_sq(np.ascontiguousarray(Wk.T)),
        "w0": _pack_sq(mem_W[0]),
        "w0T": _pack_sq(np.ascontiguousarray(mem_W[0].T)),
        "w1": _pack_sq(mem_W[1]),
        "w1T": _pack_sq(np.ascontiguousarray(mem_W[1].T)),
        "wf": _pack_gate(Wf),
        "wu": _pack_gate(Wu),
        "rows": rows,
        "bkT": np.ascontiguousarray(bk.reshape(8, 128).T).astype(np.float16),
    }
    in_maps = []
    for c in range(NC):
        m = dict(shared)
        Xc = X[c * BP:(c + 1) * BP].reshape(BP * S, D)
        # append a ones column per row so c = [X|1] . [a'|beta'] in one pass
        Xp = np.concatenate(
            [Xc, np.ones((BP * S, 1), np.float32)], axis=1)
        # DMA d, partition p, col-block j = row d*SUB*128 + j*128 + p
        Xr = Xp.reshape(NT, SUB, 128, D + 1).transpose(0, 2, 1, 3).reshape(
            NT * 128, SUB * (D + 1))
        m["x"] = np.ascontiguousarray(Xr).astype(np.float16)
        m["mem"] = np.ascontiguousarray(mem[c * BP:(c + 1) * BP])
        m["mom"] = np.ascontiguousarray(mom[c * BP:(c + 1) * BP])
        in_maps.append(m)
    return nc, in_maps


def kernel(**inputs):
    global LAST_RESULT
    nc, in_maps = _prep(inputs)
    res = run_bass_kernel_spmd(nc, in_maps, list(range(NC)))
    LAST_RESULT = res
    outs = res.results
    processed = np.concatenate([outs[c]["out_p"] for c in range(NC)], axis=0)
    new_memory = np.concatenate([outs[c]["out_m"] for c in range(NC)], axis=0)
    return processed.astype(np.float32), new_memory.astype(np.float32)


# revision 55
# speedup vs baseline: 1.2288x; 1.2288x over previous
import sys
import types

import numpy as np
from contextlib import ExitStack

try:
    import antenv.axon_hooks  # noqa: F401
except ImportError:
    _m = types.ModuleType("antenv.axon_hooks")
    _m._HOOK = None

    def _set_hook(h, _m=_m):
        _m._HOOK = h

    def _get_hook(_m=_m):
        return _m._HOOK

    _m.set_axon_ntff_profile_hook = _set_hook
    _m.get_axon_ntff_profile_hook = _get_hook
    sys.modules["antenv.axon_hooks"] = _m
    try:
        import antenv

        antenv.axon_hooks = _m
    except ImportError:
        pass

import concourse.bass as bass
import concourse.bacc as bacc
import concourse.tile as tile
from concourse import mybir
from concourse.bass_utils import run_bass_kernel_spmd
from concourse.masks import make_identity

F32 = mybir.dt.float32
F16 = mybir.dt.float16
AF = mybir.ActivationFunctionType
OP = mybir.AluOpType

B, S, D, M = 32, 2048, 1024, 1024
NC = 8
BP = B // NC          # batches per core = 4
NT = 16               # X DMA tiles per core (each [128, 4*1024] f16)
SUB = 4               # 128-row subtiles per X DMA tile
LN_EPS = 1e-5
SC = float(2 ** 20)   # scaling so the per-row gradient c stays in f16 normal range

# packed broadcast-row order (all f32, one DRAM tensor)
ROWS = ("bk", "b0", "b1", "g0", "g1", "lb0", "lb1", "bfv", "buv", "wvs")

LAST_RESULT = None    # test.py reads exec_time_ns from here


def _build(eta_f: float, theta_f: float, bvs_pre_sc: float, sc_val: float):
    nc = bacc.Bacc("TRN2", target_bir_lowering=False)
    d = nc.declare_dram_parameter
    x_d = d("x", [NT * 128, SUB * (D + 1)], F16, False)
    mem_d = d("mem", [BP, M], F32, False)
    mom_d = d("mom", [BP, M], F32, False)
    # square weights packed [128, 8*1024]: cols 1024k:1024(k+1) = W[128k:128(k+1), :]
    wkT_d = d("wkT", [128, 8 * 1024], F16, False)
    wk_d = d("wk", [128, 8 * 1024], F16, False)
    w0_d = d("w0", [128, 8 * 1024], F16, False)
    w1_d = d("w1", [128, 8 * 1024], F16, False)
    w0T_d = d("w0T", [128, 8 * 1024], F16, False)
    w1T_d = d("w1T", [128, 8 * 1024], F16, False)
    wf_d = d("wf", [128, 16 * 1024], F16, False)
    wu_d = d("wu", [128, 16 * 1024], F16, False)
    rows_d = d("rows", [BP, len(ROWS) * M], F32, False)
    bkT_d = d("bkT", [128, 8], F16, False)
    outp_d = d("out_p", [BP, M], F32, True)
    outm_d = d("out_m", [BP, M], F32, True)

    with tile.TileContext(nc) as tc, ExitStack() as ctx:
        keep = ctx.enter_context(tc.tile_pool(name="keep", bufs=1))
        temps = ctx.enter_context(tc.tile_pool(name="temps", bufs=6))
        sc = ctx.enter_context(tc.tile_pool(name="sc", bufs=12))
        wch = ctx.enter_context(tc.tile_pool(name="wch", bufs=2))
        tp = ctx.enter_context(tc.tile_pool(name="tp", bufs=3))

        def kt(tag, shape=(BP, M), dt=F32):
            return keep.tile(list(shape), dt, tag=tag, name=tag)

        def tmp():
            return temps.tile([BP, M], F32, tag="tmp", name="tmp")

        def sct():
            return sc.tile([BP, 1], F32, tag="sc", name="sc")

        ident = kt("ident", (128, 128))
        make_identity(nc, ident[:])
        epsc = kt("epsc", (BP, 1))
        nc.gpsimd.memset(epsc[:], LN_EPS)

        # ---- cached / streamed weights (halves so chunk 0 lands early) ----
        w0ca = kt("w0ca", (128, 4 * 1024), F16)
        nc.sync.dma_start(w0ca[:], w0_d[:, 0:4 * 1024])
        w0cb = kt("w0cb", (128, 4 * 1024), F16)
        nc.sync.dma_start(w0cb[:], w0_d[:, 4 * 1024:8 * 1024])
        w0c = [w0ca, w0cb]

        # broadcast rows: host replicates 4x, one direct DMA
        cbt = kt("cbt", (BP, len(ROWS) * M))
        nc.sync.dma_start(cbt[:], rows_d[:])
        cb = {n: cbt[:, i * M:(i + 1) * M] for i, n in enumerate(ROWS)}

        mem_sb = kt("mem")
        nc.sync.dma_start(mem_sb[:], mem_d[:])
        mom_sb = kt("mom")
        nc.sync.dma_start(mom_sb[:], mom_d[:])
        bkT = kt("bkT", (128, 8), F16)
        nc.sync.dma_start(bkT[:], bkT_d[:])

        def stream_w(dram, off=0):
            t = wch.tile([128, 4 * 1024], F16, tag="wch")
            nc.sync.dma_start(t[:], dram[:, off:off + 4 * 1024])
            return t

        def stream_sq(dram):
            return [stream_w(dram, 0), stream_w(dram, 4 * 1024)]

        def transpose_4(src, ps_tp, tag, dst_pool=None, cast_scale=None):
            # [4, 1024] f32 -> f16 [128, 32]; chunk k lives at cols 4k:4k+4
            pool = dst_pool if dst_pool is not None else tp
            dst = pool.tile([128, 4 * (M // 128)], F16, tag=tag)
            for k in range(M // 128):
                pt = ps_tp.tile([128, BP], F32, tag="pt")
                nc.tensor.transpose(pt[:], src[:, 128 * k:128 * (k + 1)],
                                    ident[0:BP, 0:BP])
                if cast_scale is None:
                    nc.scalar.copy(dst[:, 4 * k:4 * k + 4], pt[:])
                else:
                    nc.scalar.activation(dst[:, 4 * k:4 * k + 4], pt[:],
                                         AF.Copy, scale=cast_scale)
            return dst

        def mm_sb(lhsT_ap_fn, w_tiles, nk, ps_mm, evict):
            # out[b, n] = sum_k lhs[b, k] * W[k, n]; rhs views into resident
            # SBUF tiles (w_tiles[k] -> (tile, col_off) for chunk k)
            pz0 = ps_mm.tile([BP, 512], F32, tag="pz0")
            pz1 = ps_mm.tile([BP, 512], F32, tag="pz1")
            for k in range(nk):
                wt, off = w_tiles(k)
                nc.tensor.matmul(pz0[:], lhsT_ap_fn(k), wt[:, off:off + 512],
                                 start=(k == 0), stop=(k == nk - 1))
                nc.tensor.matmul(pz1[:], lhsT_ap_fn(k), wt[:, off + 512:off + 1024],
                                 start=(k == 0), stop=(k == nk - 1))
            evict(0, pz0)
            evict(1, pz1)

        def sq_tiles(t):
            if isinstance(t, list):
                return lambda k: (t[k // 4], 1024 * (k % 4))
            return lambda k: (t, 1024 * k)

        def layer_forward(h_sb, w_tile, b_b, g_b, lb_b, ps_tp, ps_mm, li,
                          hT_tag=None, hT_pool=None, save=False):
            hT = transpose_4(h_sb, ps_tp, hT_tag or f"hT{li}", dst_pool=hT_pool)
            z_sb = tmp()

            def ev(half, pz):
                nc.vector.tensor_add(z_sb[:, 512 * half:512 * half + 512], pz[:],
                                     b_b[:, 512 * half:512 * half + 512])

            mm_sb(lambda k: hT[:, 4 * k:4 * k + 4], sq_tiles(w_tile), 8, ps_mm, ev)

            ssum = sct()
            nc.vector.tensor_reduce(ssum[:], z_sb[:], mybir.AxisListType.X, OP.add)
            nmean = sct()
            nc.scalar.mul(nmean[:], ssum[:], -1.0 / M)
            sq = tmp()
            vs = sct()
            nc.scalar.activation(sq[:], z_sb[:], AF.Square, bias=nmean[:],
                                 accum_out=vs[:])
            std = sct()
            nc.scalar.activation(std[:], vs[:], AF.Sqrt, bias=epsc[:],
                                 scale=1.0 / M)
            rstd = kt(f"rstd{li}", (BP, 1)) if save else sct()
            nc.vector.reciprocal(rstd[:], std[:])
            xhat = kt(f"xhat{li}") if save else tmp()
            nc.vector.tensor_scalar(xhat[:], z_sb[:], nmean[:], rstd[:],
                                    OP.add, OP.mult)
            yt = tmp()
            nc.vector.tensor_mul(yt[:], xhat[:], g_b[:])
            y_sb = kt(f"y{li}") if save else tmp()
            nc.vector.tensor_add(y_sb[:], yt[:], lb_b[:])
            h_next = tmp()
            nc.scalar.activation(h_next[:], y_sb[:], AF.Silu)
            return h_next, hT, xhat, y_sb, rstd

        # ---------- Phase A: forward MLP(mem) -> mo, then u, a, beta ----------
        with tc.tile_pool(name="pstp_a", bufs=2, space="PSUM") as ps_tp, \
             tc.tile_pool(name="psmm_a", bufs=2, space="PSUM") as ps_mm, \
             tc.tile_pool(name="rowp", bufs=2) as rowp:
            w1_sa = stream_sq(w1_d)
            wkT_sb = stream_sq(wkT_d)

            h1, memT, xhat0, y0, rstd0 = layer_forward(
                mem_sb, w0c, cb["b0"], cb["g0"], cb["lb0"], ps_tp, ps_mm, 0,
                hT_tag="memT", hT_pool=keep, save=True)
            mo, _, xhat1, y1, rstd1 = layer_forward(
                h1, w1_sa, cb["b1"], cb["g1"], cb["lb1"], ps_tp, ps_mm, 1,
                save=True)

            # kappa = mo . bk via PE (moT chunks x bkT cols)
            moT = transpose_4(mo, ps_tp, "moT")
            kap = kt("kap", (BP, 1))
            kpz = ps_mm.tile([BP, 1], F32, tag="kpz")
            for k in range(8):
                nc.tensor.matmul(kpz[:], moT[:, 4 * k:4 * k + 4],
                                 bkT[:, k:k + 1], start=(k == 0), stop=(k == 7))
            nc.scalar.copy(kap[:], kpz[:])
            # u = mo @ WkT, pre-scaled: us = u * SC/(B*S)
            us = tmp()

            def ev_u(half, pz):
                nc.scalar.activation(us[:, 512 * half:512 * half + 512], pz[:],
                                     AF.Copy, scale=sc_val / (B * S))

            mm_sb(lambda k: moT[:, 4 * k:4 * k + 4], sq_tiles(wkT_sb), 8, ps_mm, ev_u)

            # abrow[:, 0:D] = a' = u*SC/(B*S) - wvs*SC/(B*S*M)   (wvs pre-scaled on host)
            # abrow[:, D]   = beta' = SC*(kappa/(B*S) - bvs/(B*S*M))
            abrow16 = kt("abrow16", (BP, D + 1), F16)
            nc.vector.tensor_sub(abrow16[:, 0:D], us[:], cb["wvs"])
            nc.scalar.activation(abrow16[:, D:D + 1], kap[:], AF.Copy,
                                 bias=-bvs_pre_sc, scale=sc_val / (B * S))

            # partition_broadcast input must start at partition 0 -> DMA-stage
            # (scalar queue: keeps the sync queue free for the X stream)
            a_bc = []
            for b in range(BP):
                row = rowp.tile([1, D + 1], F16, tag="row", name=f"row{b}")
                nc.scalar.dma_start(row[:], abrow16[b:b + 1, :])
                ab = kt(f"abc{b}", (128, D + 1), F16)
                nc.gpsimd.partition_broadcast(ab[:], row[:])
                a_bc.append(ab)

        # ---------- Phase B: stream X (f16), c' = SC*(X.a + beta) ----------
        gx_sb = kt("gx")
        xsum_sb = kt("xsum")
        csum_sb = kt("csum", (BP, 1))
        with tc.tile_pool(name="pa", bufs=2, space="PSUM") as pa_p, \
             tc.tile_pool(name="pb", bufs=2, space="PSUM") as pb_p, \
             tc.tile_pool(name="pc", bufs=2, space="PSUM") as pc_p, \
             tc.tile_pool(name="xt", bufs=3) as xt_p, \
             tc.tile_pool(name="ctp", bufs=3) as ct_p, \
             tc.tile_pool(name="c32p", bufs=3) as c32_p, \
             tc.tile_pool(name="scrp", bufs=4) as scr_p, \
             tc.tile_pool(name="stg", bufs=1) as stg_p:
            for b in range(BP):
                pa = pa_p.tile([2, 512], F32, tag="pa")
                pb = pb_p.tile([2, 512], F32, tag="pb")
                pc = pc_p.tile([2 * SUB, 2 * SUB], F32, tag="pc")
                for t in range(NT // BP):
                    di = b * (NT // BP) + t
                    xt = xt_p.tile([128, SUB * (D + 1)], F16, tag="xt")
                    nc.sync.dma_start(xt[:], x_d[di * 128:(di + 1) * 128, :])
                    ct = ct_p.tile([128, 2 * SUB], F16, tag="ct")
                    nc.any.memset(ct[:], 1.0)
                    c32 = c32_p.tile([128, SUB], F32, tag="c32")
                    for j in range(SUB):
                        subf = xt[:, j * (D + 1):(j + 1) * (D + 1)]
                        scr = scr_p.tile([128, D + 1], F16, tag="scr")
                        nc.vector.tensor_mul(scr[:], subf, a_bc[b][:])
                        scr2 = scr_p.tile([128, D + 1], F16, tag="scr")
                        nc.scalar.activation(scr2[:], scr[:], AF.Copy,
                                             accum_out=c32[:, j:j + 1])
                    nc.scalar.copy(ct[:, 1:2 * SUB:2], c32[:])
                    for j in range(SUB):
                        st = t * SUB + j
                        sub = xt[:, j * (D + 1):j * (D + 1) + D]
                        lt = ct[:, 2 * j:2 * j + 2]
                        fl = (st == 0)
                        ll = (st == 4 * SUB - 1)
                        nc.tensor.matmul(pa[:], lt, sub[:, 0:512], start=fl, stop=ll)
                        nc.tensor.matmul(pb[:], lt, sub[:, 512:1024], start=fl, stop=ll)
                    # csum via one [8,8] matmul per tile; row 0 odd cols hold
                    # per-subtile csums
                    nc.tensor.matmul(pc[:], ct[:], ct[:], start=(t == 0),
                                     stop=(t == NT // BP - 1))
                # lhsT rows: p=0 -> ones, p=1 -> c'
                stage = stg_p.tile([2, D + 2 * SUB + 1], F32, tag="stage")
                nc.scalar.copy(stage[:, 0:512], pa[:])
                nc.scalar.copy(stage[:, 512:1024], pb[:])
                nc.scalar.copy(stage[0:1, 1024:1024 + 2 * SUB], pc[0:1, :])
                nc.vector.tensor_reduce(
                    stage[0:1, D + 2 * SUB:D + 2 * SUB + 1],
                    stage[0:1, D + 1:D + 2 * SUB:2],
                    mybir.AxisListType.X, OP.add)
                nc.gpsimd.dma_start(xsum_sb[b:b + 1, :], stage[0:1, 0:D])
                nc.gpsimd.dma_start(gx_sb[b:b + 1, :], stage[1:2, 0:D])
                nc.gpsimd.dma_start(csum_sb[b:b + 1, 0:1],
                                    stage[0:1, D + 2 * SUB:D + 2 * SUB + 1])

        # ---------- Phase C: dmo, backward, gates, update, output MLP ----------
        with tc.tile_pool(name="pstp_c", bufs=2, space="PSUM") as ps_tp, \
             tc.tile_pool(name="psmm_c", bufs=2, space="PSUM") as ps_mm:
            # stream loads issued in consumption order; wch bufs=3 keeps the
            # DMA queue busy end-to-end
            wk_sb = stream_sq(wk_d)
            w1T_sb = stream_sq(w1T_d)
            w0T_sb = stream_sq(w0T_d)
            wf_sb = [stream_w(wf_d, i * 4 * 1024) for i in range(4)]
            wu_sb = [stream_w(wu_d, i * 4 * 1024) for i in range(4)]
            w1_sc = stream_sq(w1_d)



            # dmo' = SC*dmo = gx' @ Wk + csum' * bk; the 1/SC unscale is
            # deferred to the theta multiply (backward is linear in dmo)
            bkc = tmp()
            nc.vector.tensor_scalar(bkc[:], cb["bk"], csum_sb[:, 0:1], None,
                                    OP.mult)
            gxT = transpose_4(gx_sb, ps_tp, "gxT")
            dmo = kt("dmo")

            def ev_dmo(half, pz):
                nc.vector.tensor_add(dmo[:, 512 * half:512 * half + 512], pz[:],
                                     bkc[:, 512 * half:512 * half + 512])

            mm_sb(lambda k: gxT[:, 4 * k:4 * k + 4], sq_tiles(wk_sb), 8, ps_mm, ev_dmo)

            # backward through the 2-layer MLP
            dcur = dmo
            for i in (1, 0):
                y_i = y1 if i == 1 else y0
                xh_i = xhat1 if i == 1 else xhat0
                rs_i = rstd1 if i == 1 else rstd0
                g_b = cb["g1"] if i == 1 else cb["g0"]
                wT_sb = w1T_sb if i == 1 else w0T_sb

                t4 = tmp()
                nc.scalar.activation(t4[:], y_i[:], AF.Derivative_silu)
                dy = tmp()
                nc.vector.tensor_mul(dy[:], dcur[:], t4[:])
                dxh = tmp()
                nc.vector.tensor_mul(dxh[:], dy[:], g_b[:])

                rsum = sct()
                nc.vector.tensor_reduce(rsum[:], dxh[:], mybir.AxisListType.X, OP.add)
                nm1 = sct()
                nc.scalar.mul(nm1[:], rsum[:], -1.0 / M)
                junk = tmp()
                nc.vector.tensor_mul(junk[:], dxh[:], xh_i[:])
                rs2 = sct()
                junk2 = tmp()
                nc.scalar.activation(junk2[:], junk[:], AF.Copy, accum_out=rs2[:])
                nmh = sct()
                nc.scalar.mul(nmh[:], rs2[:], -1.0 / M)
                t5 = tmp()
                nc.vector.tensor_scalar(t5[:], xh_i[:], nmh[:], nm1[:],
                                        OP.mult, OP.add)
                t6 = tmp()
                nc.vector.tensor_add(t6[:], dxh[:], t5[:])
                dz = tmp()
                nc.vector.tensor_scalar(dz[:], t6[:], rs_i[:], None, OP.mult)

                # layer-0 dz is ~SC*rstd0*rstd1 scaled; shrink 2^-10 at the
                # f16 cast to stay under f16 max (compensated in theta mul)
                dzT = transpose_4(dz, ps_tp, f"dzT{i}",
                                  cast_scale=(None if i == 1 else 2.0 ** -10))
                dnext = kt(f"dh{i}")

                def ev_dh(half, pz, _dst=dnext):
                    nc.scalar.copy(_dst[:, 512 * half:512 * half + 512], pz[:])

                mm_sb(lambda k: dzT[:, 4 * k:4 * k + 4], sq_tiles(wT_sb), 8,
                      ps_mm, ev_dh)
                dcur = dnext
            surprise = dcur

            # gates: gate_in = [pooled | mem]; 1/S folded into the f16 cast
            pooledT = transpose_4(xsum_sb, ps_tp, "pooledT",
                                  cast_scale=1.0 / S)

            def gate_lhsT(k):
                if k < 8:
                    return pooledT[:, 4 * k:4 * k + 4]
                return memT[:, 4 * (k - 8):4 * (k - 8) + 4]

            def make_gate(w_tiles, bias_b, tag):
                g_sb = kt(tag)

                def gw(k):
                    return (w_tiles[k // 4], 1024 * (k % 4))

                def ev(half, pz):
                    tt = tmp()
                    nc.vector.tensor_add(tt[:, 0:512], pz[:],
                                         bias_b[:, 512 * half:512 * half + 512])
                    nc.scalar.activation(g_sb[:, 512 * half:512 * half + 512],
                                         tt[:, 0:512], AF.Sigmoid)

                mm_sb(gate_lhsT, gw, 16, ps_mm, ev)
                return g_sb

            forget_g = make_gate(wf_sb, cb["bfv"], "fgate")
            update_g = make_gate(wu_sb, cb["buv"], "ugate")

            # new_momentum = eta*mom + (theta/SC)*surprise'  (deferred unscale)
            ta = tmp()
            nc.vector.tensor_scalar(ta[:], mom_sb[:], eta_f, None, OP.mult)
            tb = tmp()
            nc.vector.tensor_scalar(tb[:], surprise[:],
                                    theta_f * (2.0 ** 10) / sc_val, None,
                                    OP.mult)
            nm_sb = tmp()
            nc.vector.tensor_add(nm_sb[:], ta[:], tb[:])

            # new_memory = (1-forget)*mem + update*new_momentum
            tc1 = tmp()
            nc.vector.tensor_mul(tc1[:], forget_g[:], mem_sb[:])
            tc2 = tmp()
            nc.vector.tensor_sub(tc2[:], mem_sb[:], tc1[:])
            tc3 = tmp()
            nc.vector.tensor_mul(tc3[:], update_g[:], nm_sb[:])
            newmem = kt("newmem")
            nc.vector.tensor_add(newmem[:], tc2[:], tc3[:])

            # processed = MLP(new_memory), weights cached in SBUF
            p1, _, _, _, _ = layer_forward(newmem, w0c, cb["b0"], cb["g0"],
                                           cb["lb0"], ps_tp, ps_mm, 0,
                                           hT_tag="nmT")
            proc, _, _, _, _ = layer_forward(p1, w1_sc, cb["b1"], cb["g1"],
                                             cb["lb1"], ps_tp, ps_mm, 1,
                                             hT_tag="p1T")

            nc.sync.dma_start(outp_d[:], proc[:])
            nc.sync.dma_start(outm_d[:], newmem[:])

    nc.finalize()
    return nc


def _pack_sq(W):
    # [1024, 1024] -> [128, 8*1024] f16, chunk k = W[128k:128(k+1), :]
    return np.ascontiguousarray(
        W.reshape(8, 128, 1024).transpose(1, 0, 2).reshape(128, 8 * 1024)
    ).astype(np.float16)


def _pack_gate(W):
    # [2048, 1024] -> [128, 16*1024] f16
    return np.ascontiguousarray(
        W.reshape(16, 128, 1024).transpose(1, 0, 2).reshape(128, 16 * 1024)
    ).astype(np.float16)


def _prep(inputs):
    f = lambda k: np.ascontiguousarray(np.asarray(inputs[k], dtype=np.float32))
    X = f("inputs")
    mem = f("memory_state")
    mom = f("momentum_state")
    Wk, bk = f("Wk"), f("bk")
    Wv, bv = f("Wv"), f("bv")
    mem_W, mem_b = f("mem_W"), f("mem_b")
    ln_g, ln_b = f("ln_g"), f("ln_b")
    Wf, Wu = f("Wf"), f("Wu")
    bfv, buv = f("bf"), f("bu")
    eta_f = float(np.asarray(inputs["eta"]).reshape(-1)[0])
    theta_f = float(np.asarray(inputs["theta"]).reshape(-1)[0])

    # keep c' = SC*c in f16 normal range: mem==0 makes c ~1e-7, else ~1e-4
    sc_val = float(2 ** 20) if float(np.abs(mem).max()) < 1e-6 else float(2 ** 10)
    bvs_pre_sc = float(bv.sum()) * sc_val / (B * S * M)
    wvs_sc = (Wv.sum(axis=1) * sc_val / (B * S * M)).astype(np.float32)

    nc = _build(eta_f, theta_f, bvs_pre_sc, sc_val)

    rowvals = {
        "bk": bk, "b0": mem_b[0], "b1": mem_b[1], "g0": ln_g[0], "g1": ln_g[1],
        "lb0": ln_b[0], "lb1": ln_b[1], "bfv": bfv, "buv": buv, "wvs": wvs_sc,
    }
    rows = np.concatenate([rowvals[n].reshape(-1) for n in ROWS]).astype(
        np.float32).reshape(1, len(ROWS) * M)
    rows = np.ascontiguousarray(np.broadcast_to(rows, (BP, len(ROWS) * M)))

    shared = {
        "wk": _pack_sq(Wk),
        "wkT": _pack_sq(np.ascontiguousarray(Wk.T)),
        "w0": _pack_sq(mem_W[0]),
        "w0T": _pack_sq(np.ascontiguousarray(mem_W[0].T)),
        "w1": _pack_sq(mem_W[1]),
        "w1T": _pack_sq(np.ascontiguousarray(mem_W[1].T)),
        "wf": _pack_gate(Wf),
        "wu": _pack_gate(Wu),
        "rows": rows,
        "bkT": np.ascontiguousarray(bk.reshape(8, 128).T).astype(np.float16),
    }
    in_maps = []
    for c in range(NC):
        m = dict(shared)
        Xc = X[c * BP:(c + 1) * BP].reshape(BP * S, D)
        # append a ones column per row so c = [X|1] . [a'|beta'] in one pass
        Xp = np.concatenate(
            [Xc, np.ones((BP * S, 1), np.float32)], axis=1)
        # DMA d, partition p, col-block j = row d*SUB*128 + j*128 + p
        Xr = Xp.reshape(NT, SUB, 128, D + 1).transpose(0, 2, 1, 3).reshape(
            NT * 128, SUB * (D + 1))
        m["x"] = np.ascontiguousarray(Xr).astype(np.float16)
        m["mem"] = np.ascontiguousarray(mem[c * BP:(c + 1) * BP])
        m["mom"] = np.ascontiguousarray(mom[c * BP:(c + 1) * BP])
        in_maps.append(m)
    return nc, in_maps


def kernel(**inputs):
    global LAST_RESULT
    nc, in_maps = _prep(inputs)
    res = run_bass_kernel_spmd(nc, in_maps, list(range(NC)))
    LAST_RESULT = res
    outs = res.results
    processed = np.concatenate([outs[c]["out_p"] for c in range(NC)], axis=0)
    new_memory = np.concatenate([outs[c]["out_m"] for c in range(NC)], axis=0)
    return processed.astype(np.float32), new_memory.astype(np.float32)
